# revision 1
# baseline (speedup 1.0000x reference)
"""Trainium2 Bass kernel for nn_BuildLstmUnrollNet (2-layer LSTM, 11-step unroll,
per-step weights), distributed over 8 NeuronCores.

Strategy: 8-way tensor parallelism over the 4*R gate dimension. Each core owns a
128-row slice of each of the 4 gates (512 of 4096 pre-activation columns), so
per-step weight reads are fully disjoint across cores (HBM traffic /8). The
full batch (256) is kept on every core. Matmuls run with activations stationary
(x^T / h^T tiles, [K=128, M=128]) and weight slices moving ([128, 512]), in
bf16 with fp32 PSUM accumulation; biases are folded in as K=1 rank-1 matmuls.

The two batch halves are fully independent recurrences, so they run as two
skewed pipelines: after each cell's gating, the half's new h slice is
PE-transposed, cast to bf16 and AllGathered ([128,128] bf16, 4 collectives per
step) while the PE runs the other half's matmuls. Cell states c stay
core-local in fp32. The scan emits the top-layer h *before* each step's
update, so only 10 of the 11 unrolled steps are computed.

Measured end-to-end rel. error ~7e-4 vs the fp32 reference (bf16 matmul
precision).
"""
import numpy as np

B, I, R, L, U = 256, 512, 1024, 2, 11
U_RUN = U - 1          # the 11th step never reaches the output
N_CORES = 8
RC = 128               # per-core rows per gate
W = 4 * RC             # per-core pre width (512)
NKX = I // 128         # 4  x   k-tiles (layer0 input part)
NKH = R // 128         # 8  h   k-tiles
NK0 = NKX + NKH        # 12 layer0 k-tiles
NK1 = 2 * NKH          # 16 layer1 k-tiles (Wh first, then Wi)
NB = B // 128          # 2 batch halves (skewed pipelines)
BH = 128               # batch half


def build_program(reps: int = 1, inline_w: dict | None = None,
                  no_collective: bool = False, ag_nowait: bool = False):
    import concourse.bacc as bacc
    import concourse.mybir as mybir
    import concourse.tile as tile
    import concourse.masks as masks

    F32 = mybir.dt.float32
    BF16 = mybir.dt.bfloat16
    Sig = mybir.ActivationFunctionType.Sigmoid
    Tanh = mybir.ActivationFunctionType.Tanh

    PHASE_MARKS.clear()
    nc = bacc.Bacc("TRN2", target_bir_lowering=False, debug=False,
                   num_devices=N_CORES)

    xT_d = nc.dram_tensor("xT", [I, B], BF16, kind="ExternalInput")
    h0T_d = nc.dram_tensor("h0T0", [R, B], BF16, kind="ExternalInput")
    h1T_d = nc.dram_tensor("h1T0", [R, B], BF16, kind="ExternalInput")
    c0_d = nc.dram_tensor("c0i", [B, RC], F32, kind="ExternalInput")
    c1_d = nc.dram_tensor("c1i", [B, RC], F32, kind="ExternalInput")
    if inline_w is None:
        wall_d = nc.dram_tensor("wall", [U_RUN, 128, (NK0 + NK1) * W], BF16,
                                kind="ExternalInput")
    else:
        # timing builds: bake the (large) weights into the NEFF so repeated
        # dispatches don't re-ship ~300MB through the axon relay
        wall_d = nc.inline_tensor(inline_w["wall"], name="wallc")
    b0_d = nc.dram_tensor("b0", [1, U_RUN * W], BF16, kind="ExternalInput")
    b1_d = nc.dram_tensor("b1", [1, U_RUN * W], BF16, kind="ExternalInput")
    y_d = nc.dram_tensor("y", [U_RUN, B, RC], F32, kind="ExternalOutput")

    with tile.TileContext(nc) as tc:
        with tc.tile_pool(name="const", bufs=1) as constp, \
             tc.tile_pool(name="hpool", bufs=2) as hpool, \
             tc.tile_pool(name="cpool", bufs=2) as cpool, \
             tc.tile_pool(name="wpool", bufs=3) as wpool, \
             tc.tile_pool(name="gpool", bufs=2) as gpool, \
             tc.tile_pool(name="stage", bufs=2) as stage, \
             tc.tile_pool(name="ps0", bufs=1, space="PSUM") as ps0, \
             tc.tile_pool(name="ps1", bufs=1, space="PSUM") as ps1, \
             tc.tile_pool(name="psT", bufs=2, space="PSUM") as psT, \
             tc.tile_pool(name="dramp", bufs=2, space="DRAM") as dramp:

            # ---- constants ----
            ident = constp.tile([128, 128], F32)
            masks.make_identity(nc, ident[:])
            ones = constp.tile([1, 128], BF16)
            nc.gpsimd.memset(ones[:], 1.0)
            xT_tiles = []
            for kk in range(NKX):
                xt = constp.tile([128, B], BF16, name=f"xT_{kk}")
                nc.sync.dma_start(xt[:], xT_d.ap()[kk * 128 : (kk + 1) * 128, :])
                xT_tiles.append(xt)
            bias0 = constp.tile([1, U_RUN * W], BF16)
            nc.sync.dma_start(bias0[:], b0_d.ap())
            bias1 = constp.tile([1, U_RUN * W], BF16)
            nc.sync.dma_start(bias1[:], b1_d.ap())

            # ---- helpers ----
            def load_h(layer, src_ap):
                """[R, B] DRAM h^T -> one [128, NKH*B] tile (k-chunk kk at
                columns [kk*B, (kk+1)*B)), as two DMAs so the first k-chunks'
                consumers start earlier."""
                t = hpool.tile([128, NKH * B], BF16, name=f"h{layer}T",
                               tag=f"h{layer}T")
                half = NKH // 2
                for lo in (0, half):
                    nc.sync.dma_start(
                        t[:, lo * B : (lo + half) * B]
                        .rearrange("p (k b) -> p k b", k=half),
                        src_ap[lo * 128 : (lo + half) * 128, :]
                        .rearrange("(k p) b -> p k b", p=128),
                    )
                return t

            def load_wall(src_ap):
                """Per-step weights as two tiles (layer0 / layer1) on the
                Activation HWDGE queue, so the big transfers never block the
                latency-critical gather hops on the SP queue."""
                ta = wpool.tile([128, NK0 * W], BF16, name="w0_s", tag="w0_s")
                for lo in range(0, NK0, 4):
                    hi = min(lo + 4, NK0)
                    nc.scalar.dma_start(ta[:, lo * W : hi * W],
                                        src_ap[:, lo * W : hi * W])
                tb = wpool.tile([128, NK1 * W], BF16, name="w1_s", tag="w1_s")
                for lo in range(0, NK1, 4):
                    hi = min(lo + 4, NK1)
                    nc.scalar.dma_start(tb[:, lo * W : hi * W],
                                        src_ap[:, (NK0 + lo) * W : (NK0 + hi) * W])
                return ta, tb

            def gate_cell(layer, b, pre_ps, c_old, hn_tile):
                """LSTM cell elementwise part; writes h into hn_tile[:, b*RC:]
                and returns c_new."""
                sig = gpool.tile([128, 3 * RC], F32, name=f"sig{layer}_{b}",
                                 tag=f"sig{layer}_{b}")
                nc.scalar.activation(sig[:], pre_ps[:, : 3 * RC], Sig)
                tg = gpool.tile([128, RC], F32, name=f"tg{layer}_{b}",
                                tag=f"tg{layer}_{b}")
                nc.scalar.activation(tg[:], pre_ps[:, 3 * RC :], Tanh)
                t1 = gpool.tile([128, RC], F32, name=f"t1{layer}_{b}",
                                tag=f"t1{layer}_{b}")
                nc.vector.tensor_mul(t1[:], sig[:, :RC], tg[:])
                t2 = gpool.tile([128, RC], F32, name=f"t2{layer}_{b}",
                                tag=f"t2{layer}_{b}")
                nc.vector.tensor_mul(t2[:], sig[:, RC : 2 * RC], c_old[:])
                cnew = cpool.tile([128, RC], F32, name=f"c{layer}_{b}",
                                  tag=f"c{layer}_{b}")
                nc.vector.tensor_add(cnew[:], t1[:], t2[:])
                tcc = gpool.tile([128, RC], F32, name=f"tc{layer}_{b}",
                                 tag=f"tc{layer}_{b}")
                nc.scalar.activation(tcc[:], cnew[:], Tanh)
                nc.vector.tensor_mul(hn_tile[:, b * RC : (b + 1) * RC],
                                     sig[:, 2 * RC : 3 * RC], tcc[:])
                return cnew

            def gather_launch(layer, hn_tile):
                """PE-transpose both halves of the h slice, cast bf16, kick one
                AllGather; returns the gathered DRAM tile (landed later)."""
                tps = psT.tile([128, B], F32, name=f"tps{layer}", tag="tps")
                for b in range(NB):
                    nc.tensor.transpose(tps[:, b * BH : (b + 1) * BH],
                                        hn_tile[:, b * RC : (b + 1) * RC],
                                        ident[:])
                hst = stage.tile([128, B], BF16, name=f"hst{layer}",
                                 tag=f"hst{layer}")
                nc.scalar.copy(hst[:], tps[:])
                ag_in = dramp.tile([128, B], BF16, name=f"agin{layer}",
                                   tag=f"agin{layer}")
                nc.sync.dma_start(ag_in[:], hst[:])
                ag_out = dramp.tile(
                    [R, B], BF16, name=f"agout{layer}",
                    tag=f"agout{layer}",
                    addr_space="Local" if no_collective else "Shared")
                if no_collective:
                    # timing proxy: 2-hop DRAM chain ~ the measured ~3us AG
                    # latency, without cross-core deps
                    tmp = dramp.tile([128, B], BF16, name=f"agt{layer}",
                                     tag=f"agt{layer}")
                    nc.sync.dma_start(tmp[:], ag_in[:])
                    nc.sync.dma_start(ag_out[:][0:128, :], tmp[:])
                else:
                    nc.gpsimd.collective_compute(
                        "AllGather", mybir.AluOpType.bypass,
                        replica_groups=[list(range(N_CORES))],
                        ins=[ag_in[:]], outs=[ag_out[:]],
                    )
                return ag_out

            def gather_land(layer, ag_out):
                if ag_nowait:
                    # timing diagnostic: don't wait for the gather — land from
                    # the (constant) initial-h input instead
                    return load_h(layer, (h0T_d if layer == 0 else h1T_d).ap())
                return load_h(layer, ag_out[:])

            def hstat(h_s, kk, b):
                return h_s[:, kk * B + b * BH : kk * B + (b + 1) * BH]

            def mm(p, lhsT, wslice, start, stop):
                nc.tensor.matmul(p[:], lhsT, wslice, start=start, stop=stop)

            def pre0_start(widx, wall_s, b):
                """pre0 = bias + x part (weight-step widx), half b."""
                w0_s, _ = wall_s
                p = ps0.tile([128, W], F32, name=f"pre0_{b}", tag=f"pre0_{b}")
                mm(p, ones[:], bias0[:, widx * W : (widx + 1) * W], True, False)
                for kk in range(NKX):
                    mm(p, xT_tiles[kk][:, b * BH : (b + 1) * BH],
                       w0_s[:, kk * W : (kk + 1) * W], False, False)
                return p

            def pre0_finish(p, h0T_s, b, wall_s):
                w0_s, _ = wall_s
                for kh in range(NKH):
                    mm(p, hstat(h0T_s, kh, b),
                       w0_s[:, (NKX + kh) * W : (NKX + kh + 1) * W],
                       False, kh == NKH - 1)

            def pre1_start(widx, wall_s, h1T_s, b):
                """pre1 = bias + Wh1 part (old h1), half b."""
                _, w1_s = wall_s
                p = ps1.tile([128, W], F32, name=f"pre1_{b}", tag=f"pre1_{b}")
                mm(p, ones[:], bias1[:, widx * W : (widx + 1) * W], True, False)
                for kh in range(NKH):
                    mm(p, hstat(h1T_s, kh, b),
                       w1_s[:, kh * W : (kh + 1) * W],
                       False, False)
                return p

            def pre1_finish(p, h0T_s, b, wall_s):
                _, w1_s = wall_s
                for kh in range(NKH):
                    mm(p, hstat(h0T_s, kh, b),
                       w1_s[:, (NKH + kh) * W : (NKH + kh + 1) * W],
                       False, kh == NKH - 1)

            # ---- prologue ----
            h0T = load_h(0, h0T_d.ap())
            h1T = load_h(1, h1T_d.ap())
            c = {}
            for layer, cd in ((0, c0_d), (1, c1_d)):
                for b in range(NB):
                    t = cpool.tile([128, RC], F32, name=f"c{layer}_{b}",
                                   tag=f"c{layer}_{b}")
                    nc.sync.dma_start(t[:], cd.ap()[b * 128 : (b + 1) * 128, :])
                    c[layer, b] = t

            wall_s = load_wall(wall_d.ap()[0])
            wall_n = load_wall(wall_d.ap()[1])
            # step-0 layer0 fully in prologue (the loop body starts at pre1)
            pre0_pend = [pre0_start(0, wall_s, b) for b in range(NB)]
            for b in range(NB):
                pre0_finish(pre0_pend[b], h0T, b, wall_s)
            h0new = gpool.tile([128, NB * RC], F32, name="h0new", tag="h0new")
            for b in range(NB):
                c[0, b] = gate_cell(0, b, pre0_pend[b], c[0, b], h0new)
            ag0 = gather_launch(0, h0new)
            h0T = gather_land(0, ag0)

            # ---- steps ----
            # loop body for step t:
            #   pre1wh(t) | xpart(t+1) | pre1wi(t) {needs AG0(t)} | gate1(t)+y
            #   | T1/AG1(t) | pre0h(t+1) | gate0(t+1) | T0/AG0(t+1)
            for rep in range(reps):
                for t in range(U_RUN):
                    widx = t
                    nidx = (t + 1) % U_RUN
                    has_next = not (rep == reps - 1 and t == U_RUN - 1)
                    write_y = rep == 0

                    if has_next:
                        _mark(nc, f"r{rep}t{t}:wdma")
                        wall_n2 = load_wall(wall_d.ap()[(t + 2) % U_RUN])
                        _mark(nc, f"r{rep}t{t}:xpart")
                        pre0_pend = [pre0_start(nidx, wall_n, b)
                                     for b in range(NB)]

                    _mark(nc, f"r{rep}t{t}:pre1wh")
                    pre1 = [pre1_start(widx, wall_s, h1T, b)
                            for b in range(NB)]

                    _mark(nc, f"r{rep}t{t}:pre1wi")
                    for b in range(NB):
                        pre1_finish(pre1[b], h0T, b, wall_s)

                    _mark(nc, f"r{rep}t{t}:gate1")
                    h1new = gpool.tile([128, NB * RC], F32, name="h1new",
                                       tag="h1new")
                    for b in range(NB):
                        c[1, b] = gate_cell(1, b, pre1[b], c[1, b], h1new)
                    if write_y:
                        nc.sync.dma_start(
                            y_d.ap()[t].rearrange("(nb p) r -> p nb r", p=128),
                            h1new[:].rearrange("p (nb r) -> p nb r", nb=NB))

                    if has_next:
                        _mark(nc, f"r{rep}t{t}:tg1")
                        ag1 = gather_launch(1, h1new)

                        _mark(nc, f"r{rep}t{t}:pre0h")
                        for b in range(NB):
                            pre0_finish(pre0_pend[b], h0T, b, wall_n)
                        

                        _mark(nc, f"r{rep}t{t}:gate0")
                        h0new = gpool.tile([128, NB * RC], F32, name="h0new",
                                           tag="h0new")
                        for b in range(NB):
                            c[0, b] = gate_cell(0, b, pre0_pend[b], c[0, b],
                                                h0new)

                        _mark(nc, f"r{rep}t{t}:tg0")
                        ag0 = gather_launch(0, h0new)
                        h1T = gather_land(1, ag1)
                        h0T = gather_land(0, ag0)

                        wall_s = wall_n
                        wall_n = wall_n2

    _mark(nc, "end")
    nc.compile()
    return nc


def prepare_in_maps(inputs: dict) -> list[dict]:
    import ml_dtypes
    bf = ml_dtypes.bfloat16

    x = np.ascontiguousarray(np.asarray(inputs["x"], np.float32))
    st = np.asarray(inputs["init_states_input"], np.float32).reshape(B, 2 * L, R)
    h0i, c0i, h1i, c1i = st[:, 0], st[:, 1], st[:, 2], st[:, 3]

    xT = x.T.astype(bf)
    h0T = h0i.T.astype(bf)
    h1T = h1i.T.astype(bf)

    Wi0 = np.asarray(inputs["Wi0"], np.float32)[:U_RUN]
    Wh0 = np.asarray(inputs["Wh0"], np.float32)[:U_RUN]
    Wi1 = np.asarray(inputs["Wi1"], np.float32)[:U_RUN]
    Wh1 = np.asarray(inputs["Wh1"], np.float32)[:U_RUN]
    b0_full = (np.asarray(inputs["bi0"], np.float32)
               + np.asarray(inputs["bh0"], np.float32))[:U_RUN]
    b1_full = (np.asarray(inputs["bi1"], np.float32)
               + np.asarray(inputs["bh1"], np.float32))[:U_RUN]

    in_maps = []
    for k in range(N_CORES):
        rows = np.concatenate(
            [np.arange(g * R + k * RC, g * R + (k + 1) * RC) for g in range(4)])
        # moving weights, one tensor per step, partition-major so the per-step
        # DMA is one linear ~28KB run per partition. k-tile order:
        # [Wi0 (4) | Wh0 (8) | Wh1 (8) | Wi1 (8)], each k-tile [128, W].
        wk = np.concatenate(
            [Wi0[:, rows, :].transpose(0, 2, 1),
             Wh0[:, rows, :].transpose(0, 2, 1),
             Wh1[:, rows, :].transpose(0, 2, 1),
             Wi1[:, rows, :].transpose(0, 2, 1)], axis=1)
        wall = np.ascontiguousarray(
            wk.reshape(U_RUN, NK0 + NK1, 128, W).transpose(0, 2, 1, 3)
            .reshape(U_RUN, 128, (NK0 + NK1) * W)).astype(bf)
        in_maps.append({
            "xT": xT,
            "h0T0": h0T,
            "h1T0": h1T,
            "c0i": np.ascontiguousarray(c0i[:, k * RC : (k + 1) * RC]),
            "c1i": np.ascontiguousarray(c1i[:, k * RC : (k + 1) * RC]),
            "wall": wall,
            "b0": np.ascontiguousarray(b0_full[:, rows].reshape(1, -1)).astype(bf),
            "b1": np.ascontiguousarray(b1_full[:, rows].reshape(1, -1)).astype(bf),
        })
    return in_maps


def assemble_output(inputs: dict, results: list[dict]) -> np.ndarray:
    st = np.asarray(inputs["init_states_input"], np.float32).reshape(B, 2 * L, R)
    h1i = st[:, 2]
    out = np.empty((B, U * R), np.float32)
    out[:, :R] = h1i
    for k in range(N_CORES):
        y = results[k]["y"]  # [U_RUN, B, RC]
        for s in range(U_RUN):
            out[:, (s + 1) * R + k * RC : (s + 1) * R + (k + 1) * RC] = y[s]
    return out


PHASE_MARKS: list = []  # (label, first_instruction_id) — for profiling tools


def _mark(nc, label):
    PHASE_MARKS.append((label, nc.next_id()))


_CACHE: dict = {}


def _get_compiled():
    if "nc" not in _CACHE:
        _CACHE["nc"] = build_program(reps=1)
    return _CACHE["nc"]


def kernel(**inputs) -> np.ndarray:
    from concourse.bass_utils import run_bass_kernel_spmd

    nc = _get_compiled()
    in_maps = prepare_in_maps(inputs)
    res = run_bass_kernel_spmd(nc, in_maps, list(range(N_CORES)))
    return assemble_output(inputs, res.results)

